# revision 48
# baseline (speedup 1.0000x reference)
"""BotRGCN Trainium2 kernel: feature transform + 2 RGCN layers + classifier.

Sharding: nodes split across 8 cores by id (12500/core, padded to 12544).
Within each core, nodes are assigned to 256-wide dst windows by a balancing
greedy (equalizes per-(rel, src-bank, window) edge counts, minimizing block
padding). Edges partitioned by destination shard; per (relation, dst-window,
src-bank) groups padded to a block structure uniform across cores so a single
SPMD program serves all 8 cores. Source features exchanged via bf16 AllGather
of the per-layer node-feature table; gathers via int16 dma_gather per src
bank, round-robined across 4 SWDGE queues.
"""

import sys

sys.path.insert(0, "/opt/trn_rl_repo")

from contextlib import ExitStack

import numpy as np
import ml_dtypes

import concourse.bass as bass
import concourse.bacc as bacc
import concourse.mybir as mybir
import concourse.tile as tile
from concourse.masks import make_identity
from concourse.bass_utils import run_bass_kernel_spmd

BF16 = mybir.dt.bfloat16
F32 = mybir.dt.float32
I16 = mybir.dt.int16

P = 128

# full-problem config (test.py overrides for mini runs)
CFG = dict(
    N=100000,        # nodes
    NC=8,            # cores
    R=2,             # relations
    H=128,
    DES=768, TWEET=768, NUMP=6, CATP=11,
    WIN=128,         # dst window (PSUM free dim)
    NBLK_CH=12,      # gather-chunk size in 128-edge blocks
    EXTRA_WIN=2,     # slack windows so groups fit their ceil block count
    NTF=512,         # feature-stage node tile
    NSWQ=4,          # SWDGE queues for gather desc-gen
)


def _derived(cfg):
    d = dict(cfg)
    d["SH"] = cfg["N"] // cfg["NC"]
    shp0 = ((d["SH"] + P - 1) // P) * P
    shp0 = ((shp0 + cfg["WIN"] - 1) // cfg["WIN"]) * cfg["WIN"]
    d["SHP"] = shp0 + cfg["EXTRA_WIN"] * cfg["WIN"]
    d["NW"] = d["SHP"] // cfg["WIN"]
    d["BANKROWS"] = 2 * d["SHP"]                # bank == src core pair
    assert d["BANKROWS"] < 2 ** 15
    d["TROWS"] = cfg["NC"] * d["SHP"]           # padded table rows
    d["BANKS"] = (d["TROWS"] + d["BANKROWS"] - 1) // d["BANKROWS"]
    d["TBLK"] = d["SHP"] // P                   # 128-row blobs per core
    # x feature layout: [des | tweet | num(pad to 128) | cat(pad to 128)]
    d["KDES"] = cfg["DES"] // P
    d["KTWEET"] = cfg["TWEET"] // P
    d["KX"] = d["KDES"] + d["KTWEET"] + 2
    d["XROWS"] = d["KX"] * P
    return d


# ---------------------------------------------------------------------------
# host-side graph planning
# ---------------------------------------------------------------------------

class Plan:
    pass


def _balance_windows(deg, NW, WIN, cap):
    """Greedy node->window assignment equalizing per-dim load.

    deg: [SH, D] per-node degree vectors; cap: [D] per-dim soft cap per
    window (whole blocks). Returns slot id per node (window w slots are
    [w*WIN, (w+1)*WIN))."""
    SH = deg.shape[0]
    order = np.argsort(-deg.sum(1), kind="stable")
    load = np.zeros((NW, deg.shape[1]), np.int64)
    nslots = np.zeros(NW, np.int64)
    slot = np.zeros(SH, np.int64)
    for v in order:
        d = deg[v]
        nl = load + d
        over = np.maximum(nl - cap, 0).sum(1).astype(np.float64)
        sc = over * 1e9 + nl.max(1) + nslots * 1e-6
        sc[nslots >= WIN] = 1e18
        w = int(np.argmin(sc))
        slot[v] = w * WIN + nslots[w]
        load[w] += d
        nslots[w] += 1
    return slot


def build_plan(edge_index, edge_type, cfg):
    """Group edges per core by (rel, dst-window, src-bank); pad each group to a
    whole number of 128-edge blocks, uniform across cores. Returns per-core
    gather-index / meta arrays plus the uniform block structure."""
    d = cfg
    NC, SH, SHP, WIN, NW = d["NC"], d["SH"], d["SHP"], d["WIN"], d["NW"]
    BANKS, BR, NBLK_CH = d["BANKS"], d["BANKROWS"], d["NBLK_CH"]
    R = d["R"]
    N = d["N"]
    TBLK = d["TBLK"]

    src = np.asarray(edge_index[0], dtype=np.int64)
    dst = np.asarray(edge_index[1], dtype=np.int64)
    et = np.asarray(edge_type, dtype=np.int64)

    core = dst // SH
    dl = dst - core * SH
    score = src // SH                 # src core
    sbank = score // max(1, (BR // SHP))   # src bank == src core pair

    # --- balanced window assignment per core (slot of each local node) ---
    D = R * BANKS
    node_slot = np.zeros((NC, SH), np.int64)
    for c in range(NC):
        m = core == c
        deg = np.zeros((SH, D), np.int32)
        np.add.at(deg, (dl[m], et[m] * BANKS + sbank[m]), 1)
        cap = ((np.ceil(deg.sum(0) / NW / P)) * P).astype(np.int64)
        node_slot[c] = _balance_windows(deg, NW, WIN, cap)

    # dst slot / window / in-window pos
    dslot = node_slot[core, dl]
    win = dslot // WIN
    dw = (dslot - win * WIN).astype(np.float32)

    # src table row: slot-major (row = core*SHP + slot)
    sl = node_slot[score, src - score * SH]
    ps = score * SHP + sl
    bank = ps // BR
    bidx = (ps - bank * BR).astype(np.int16)
    assert np.all(bank == sbank)

    # per-(rel, node) in-degree -> per-edge mean weight
    cnt = np.bincount(et * N + dst, minlength=R * N).reshape(R, N)
    wv = (1.0 / np.maximum(cnt, 1.0))[et, dst].astype(np.float32)

    # group = (rel, bank, win); uniform block counts = max over cores
    NG = R * BANKS * NW
    gid = (et * BANKS + bank) * NW + win
    counts = np.bincount(core * NG + gid, minlength=NC * NG).reshape(NC, NG)
    bpg = (counts.max(axis=0) + P - 1) // P            # blocks per group
    bpg = bpg.reshape(R, BANKS, NW).copy()

    slots_per_group = (bpg.reshape(-1) * P)
    slot_base = np.zeros(NG + 1, np.int64)
    np.cumsum(slots_per_group, out=slot_base[1:])
    TOTSLOT = int(slot_base[-1])
    TOTBLK = TOTSLOT // P

    # place each edge into its group's slot range (per core)
    okey = core * NG + gid
    order = np.argsort(okey, kind="stable")
    so = okey[order]
    first_of = np.r_[True, so[1:] != so[:-1]]
    idx_in_run = np.arange(len(so)) - np.maximum.accumulate(
        np.where(first_of, np.arange(len(so)), 0)
    )
    slot = slot_base[so % NG] + idx_in_run

    # per-block dst-in-window position (bf16, -1 marks pad slots)
    idx16 = np.zeros((NC, 8 * 16, TOTSLOT // 16), np.int16)
    dwb = np.full((NC, P, TOTBLK), -1.0, ml_dtypes.bfloat16)
    ecore = core[order]
    col = slot // 16
    prow = (slot % 16).astype(np.int64)
    for g in range(8):
        idx16[ecore, 16 * g + prow, col] = bidx[order]
    dwb[ecore, slot % P, slot // P] = dw[order]

    # per-(rel, dst-slot) mean reciprocal, bf16 row vector per core
    recip = np.ones((NC, 1, R * SHP), np.float32)
    for c in range(NC):
        nd = np.arange(SH) + c * SH
        for r in range(R):
            recip[c, 0, r * SHP + node_slot[c]] = \
                1.0 / np.maximum(cnt[r, nd], 1.0)
    recip = recip.astype(ml_dtypes.bfloat16)

    pl = Plan()
    pl.idx16 = idx16.reshape(NC, P, TOTSLOT // 16)
    pl.dwb = dwb
    pl.recip = recip
    pl.bpg = bpg
    pl.TOTBLK = TOTBLK
    pl.node_slot = node_slot
    # stream bookkeeping: stream s = r*BANKS+b; block base per stream; block
    # base per group within stream
    pl.group_blk_base = np.zeros((R, BANKS, NW), np.int64)
    base = 0
    pl.stream_blk_base = np.zeros((R, BANKS), np.int64)
    pl.stream_nblk = np.zeros((R, BANKS), np.int64)
    for r in range(R):
        for b in range(BANKS):
            pl.stream_blk_base[r, b] = base
            for w in range(NW):
                pl.group_blk_base[r, b, w] = base
                base += int(bpg[r, b, w])
            pl.stream_nblk[r, b] = base - pl.stream_blk_base[r, b]
    assert base == TOTBLK
    return pl


def prep_x(x, pl, cfg):
    """Per-core transposed bf16 feature blocks [XROWS, SHP], columns in slot
    order."""
    d = cfg
    NC, SH, SHP = d["NC"], d["SH"], d["SHP"]
    NUMP, TWEET, CATP, DES = d["NUMP"], d["TWEET"], d["CATP"], d["DES"]
    KD, KT = d["KDES"], d["KTWEET"]
    out = np.zeros((NC, d["XROWS"], SHP), ml_dtypes.bfloat16)
    for c in range(NC):
        xs = x[c * SH:(c + 1) * SH]
        xT = np.zeros((d["XROWS"], SHP), np.float32)
        sl = pl.node_slot[c]
        xT[:DES, sl] = xs[:, NUMP + TWEET + CATP:].T
        xT[DES:DES + TWEET, sl] = xs[:, NUMP:NUMP + TWEET].T
        xT[(KD + KT) * P:(KD + KT) * P + NUMP, sl] = xs[:, :NUMP].T
        xT[(KD + KT + 1) * P:(KD + KT + 1) * P + CATP, sl] = \
            xs[:, NUMP + TWEET:NUMP + TWEET + CATP].T
        out[c] = xT.astype(ml_dtypes.bfloat16)
    return out


def prep_weights(inp, cfg):
    """bf16 weight blocks + packed fp32 biases."""
    bf = lambda a: np.asarray(a, np.float32).astype(ml_dtypes.bfloat16)
    d = cfg
    wnum = np.zeros((P, d["H"]), np.float32)
    wnum[:d["NUMP"]] = inp["W_num"]
    wcat = np.zeros((P, d["H"]), np.float32)
    wcat[:d["CATP"]] = inp["W_cat"]
    w = {
        "wdes": bf(inp["W_des"]), "wtweet": bf(inp["W_tweet"]),
        "wnum": bf(wnum), "wcat": bf(wcat), "win": bf(inp["W_in"]),
        "root1": bf(inp["root1"]), "rel10": bf(inp["rel1"][0]),
        "rel11": bf(inp["rel1"][1]),
        "root2": bf(inp["root2"]), "rel20": bf(inp["rel2"][0]),
        "rel21": bf(inp["rel2"][1]), "wcls": bf(inp["W_cls"]),
    }
    biases = np.stack(
        [inp["b_des"], inp["b_tweet"], inp["b_num"], inp["b_cat"],
         inp["b_in"], inp["prelu_a"], inp["bias1"], inp["bias2"],
         inp["b_cls"]], axis=1).astype(np.float32)   # [128, 9]
    w["biases"] = biases
    return w


# ---------------------------------------------------------------------------
# bass program
# ---------------------------------------------------------------------------

def build_bass(cfg, pl):
    d = cfg
    NC, SHP, WIN, NW, NTF = d["NC"], d["SHP"], d["WIN"], d["NW"], d["NTF"]
    BANKS, BR, NBLK_CH = d["BANKS"], d["BANKROWS"], d["NBLK_CH"]
    R, H = d["R"], d["H"]
    KD, KT, KX = d["KDES"], d["KTWEET"], d["KX"]
    TBLK = d["TBLK"]
    TROWS = d["TROWS"]
    NSWQ = d["NSWQ"]
    CHS = NBLK_CH * P      # idx slots per chunk

    nc = bacc.Bacc(None, target_bir_lowering=False, debug=False,
                   num_devices=NC, num_swdge_queues=NSWQ)

    # ---- I/O ----
    xT = nc.dram_tensor("xT", [d["XROWS"], SHP], BF16, kind="ExternalInput")
    idxt = nc.dram_tensor("idxt", [P, pl.TOTBLK * P // 16], I16, kind="ExternalInput")
    dwbt = nc.dram_tensor("dwbt", [P, pl.TOTBLK], BF16, kind="ExternalInput")
    recipt = nc.dram_tensor("recipt", [1, R * SHP], BF16, kind="ExternalInput")
    wts = {}
    for nm, shp in [("wdes", [d["DES"], H]), ("wtweet", [d["TWEET"], H]),
                    ("wnum", [P, H]), ("wcat", [P, H]), ("win", [4 * P, H]),
                    ("root1", [H, H]), ("rel10", [H, H]), ("rel11", [H, H]),
                    ("root2", [H, H]), ("rel20", [H, H]), ("rel21", [H, H]),
                    ("wcls", [H, H])]:
        wts[nm] = nc.dram_tensor(nm, shp, BF16, kind="ExternalInput")
    biases = nc.dram_tensor("biases", [P, 9], F32, kind="ExternalInput")
    outT = nc.dram_tensor("outT", [P, SHP], F32, kind="ExternalOutput")

    # ---- collective tables ----
    cc_in = [nc.dram_tensor(f"cc{i}_in", [SHP, H], BF16, kind="Internal")
             for i in (1, 2)]
    cc_out = [nc.dram_tensor(f"cc{i}_out", [NC * SHP, H], BF16,
                             kind="Internal", addr_space="Shared")
              for i in (1, 2)]

    rg = [list(range(NC))]
    qctr = [0]  # SWDGE queue rotation

    def cc_views(cc_in_t, cc_out_t, row0, row1):
        """Input slice + rank-strided output view for a chunked AllGather."""
        iv = cc_in_t[row0:row1, :]
        ov = cc_out_t.rearrange("(c s) h -> c s h", c=NC)[:, row0:row1, :]
        return iv, ov

    with tile.TileContext(nc) as tc:
        with (
            tc.tile_pool(name="const", bufs=1) as cpool,
            tc.tile_pool(name="resident", bufs=1) as rpool,
            ExitStack() as mstack,
        ):
            # ---- constants ----
            ident = cpool.tile([P, P], BF16)
            make_identity(nc, ident[:])
            iota = cpool.tile([P, WIN], BF16)
            nc.gpsimd.iota(iota[:], pattern=[[1, WIN]], base=0,
                           channel_multiplier=0,
                           allow_small_or_imprecise_dtypes=True)
            bias_t = cpool.tile([P, 9], F32)
            nc.sync.dma_start(out=bias_t[:], in_=biases[:])
            recip_t = cpool.tile([1, R * SHP], BF16)
            nc.sync.dma_start(out=recip_t[:], in_=recipt[:])
            ones1 = cpool.tile([1, P], BF16)
            nc.vector.memset(ones1[:], 1.0)

            wt = {}
            for nm, kb in [("wdes", KD), ("wtweet", KT), ("wnum", 1),
                           ("wcat", 1), ("win", 4), ("root1", 1),
                           ("rel10", 1), ("rel11", 1), ("root2", 1),
                           ("rel20", 1), ("rel21", 1), ("wcls", 1)]:
                t = cpool.tile([P, kb, H], BF16, tag=f"w_{nm}", name=f"w_{nm}")
                nc.sync.dma_start(
                    out=t[:], in_=wts[nm].rearrange("(k p) h -> p k h", p=P))
                wt[nm] = t

            # resident activations (transposed, [H, SHP] bf16)
            hT = [rpool.tile([P, SHP], BF16, tag="ht", name=f"hT{i}", bufs=2)
                  for i in range(3)]

            spool = mstack.enter_context(tc.tile_pool(name="tstage", bufs=4))

            def emit_table_slice(src_hT, cc_in_t, n0, n1, on_vec=False,
                                 pool=None, tbufs=1):
                """Transpose hT[:, n0:n1] into node-major rows + DMA to
                cc_in."""
                b0, b1 = n0 // P, n1 // P
                stage = spool.tile([P, 4, P], BF16, tag="tstage", name="tstage")
                for blk in range(b0, b1):
                    tp = pool.tile([P, P], BF16, tag="tp", name="tp",
                                   space="PSUM", bufs=tbufs)
                    nc.tensor.transpose(
                        out=tp[:], in_=src_hT[:, blk * P:(blk + 1) * P],
                        identity=ident[:])
                    if on_vec:
                        nc.vector.tensor_copy(out=stage[:, blk - b0, :],
                                              in_=tp[:])
                    else:
                        nc.scalar.copy(out=stage[:, blk - b0, :], in_=tp[:])
                nc.sync.dma_start(
                    out=cc_in_t.rearrange("(t p) h -> p t h", p=P)[:, b0:b1],
                    in_=stage[:, :b1 - b0])

            # =============== feature transform ===============
            fstack = ExitStack()
            fpool = fstack.enter_context(tc.tile_pool(name="featsb", bufs=4))
            fpp = fstack.enter_context(
                tc.tile_pool(name="featps", bufs=2, space="PSUM"))
            ntiles = (SHP + NTF - 1) // NTF
            for t in range(ntiles):
                n0 = t * NTF
                n1 = min(SHP, n0 + NTF)
                nn = n1 - n0
                xt = fpool.tile([P, KX, NTF], BF16, tag="xt", name="xt")
                xv = xT.rearrange("(k p) n -> p k n", p=P)
                # per-branch loads so branch matmuls start as columns arrive
                for ks, kn in [(0, KD), (KD, KT), (KD + KT, 2)]:
                    nc.sync.dma_start(
                        out=xt[:, ks:ks + kn, :nn],
                        in_=xv[:, ks:ks + kn, n0:n1])

                zb = []
                for bi, (wnm, ks, kn) in enumerate([
                        ("wdes", 0, KD), ("wtweet", KD, KT),
                        ("wnum", KD + KT, 1), ("wcat", KD + KT + 1, 1)]):
                    pz = fpp.tile([P, NTF], F32, tag=f"pz{bi}", name=f"pz{bi}", space="PSUM", bufs=1)
                    for k in range(kn):
                        nc.tensor.matmul(
                            out=pz[:, :nn], lhsT=wt[wnm][:, k, :],
                            rhs=xt[:, ks + k, :nn],
                            start=(k == 0), stop=(k == kn - 1))
                    v = fpool.tile([P, NTF], BF16, tag=f"v{bi}", name=f"v{bi}")
                    nc.scalar.activation(
                        out=v[:, :nn], in_=pz[:, :nn],
                        func=mybir.ActivationFunctionType.Identity,
                        bias=bias_t[:, bi:bi + 1])
                    z = fpool.tile([P, NTF], BF16, tag=f"z{bi}", name=f"z{bi}")
                    nc.vector.scalar_tensor_tensor(
                        out=z[:, :nn], in0=v[:, :nn], scalar=0.01,
                        in1=v[:, :nn], op0=mybir.AluOpType.mult,
                        op1=mybir.AluOpType.max)
                    zb.append(z)

                ph = fpp.tile([P, NTF], F32, tag="ph", name="ph", space="PSUM")
                for k in range(4):
                    nc.tensor.matmul(out=ph[:, :nn], lhsT=wt["win"][:, k, :],
                                     rhs=zb[k][:, :nn],
                                     start=(k == 0), stop=(k == 3))
                vh = fpool.tile([P, NTF], F32, tag="vh", name="vh")
                nc.scalar.activation(
                    out=vh[:, :nn], in_=ph[:, :nn],
                    func=mybir.ActivationFunctionType.Identity,
                    bias=bias_t[:, 4:5])
                nc.vector.scalar_tensor_tensor(
                    out=hT[0][:, n0:n1], in0=vh[:, :nn],
                    scalar=bias_t[:, 5:6], in1=vh[:, :nn],
                    op0=mybir.AluOpType.mult, op1=mybir.AluOpType.max)
                emit_table_slice(hT[0], cc_in[0], n0, n1, on_vec=True,
                                 pool=fpp, tbufs=2)

            fstack.close()
            tpool = mstack.enter_context(
                tc.tile_pool(name="tps", bufs=2, space="PSUM"))
            nc.gpsimd.collective_compute(
                "AllGather", mybir.AluOpType.bypass,
                ins=[cc_in[0][:]], outs=[cc_out[0][:]], replica_groups=rg)

            wpool = mstack.enter_context(tc.tile_pool(name="work", bufs=3))
            ppool = mstack.enter_context(
                tc.tile_pool(name="psum", bufs=2, space="PSUM"))

            # =============== per-layer helper ===============
            def emit_layer(li, h_in, h_out, table, rootw, relw, bias_col,
                           table_out=None, cls=False):
                # per-stream gather state
                cur = {}
                CB = 16
                cls_done = [0]

                def emit_cls(upto):
                    for wc in range(cls_done[0], upto):
                        wcs = slice(wc * WIN, (wc + 1) * WIN)
                        pc = ppool.tile([P, WIN], F32, tag="po", name="pc",
                                        space="PSUM", bufs=2)
                        nc.tensor.matmul(out=pc[:], lhsT=wt["wcls"][:, 0, :],
                                         rhs=h_out[:, wcs], start=True,
                                         stop=True)
                        oc = wpool.tile([P, WIN], F32, tag="oc", name="oc",
                                        bufs=2)
                        nc.scalar.activation(
                            out=oc[:], in_=pc[:],
                            func=mybir.ActivationFunctionType.Identity,
                            bias=bias_t[:, 8:9])
                        nc.sync.dma_start(out=outT[:, wcs], in_=oc[:])
                    cls_done[0] = upto

                def ensure_chunk(r, b, blkloc):
                    ch = blkloc // NBLK_CH
                    key = (r, b)
                    if cur.get(key, (-1,))[0] == ch:
                        return cur[key]
                    nblk = min(NBLK_CH,
                               int(pl.stream_nblk[r, b]) - ch * NBLK_CH)
                    gblk0 = int(pl.stream_blk_base[r, b]) + ch * NBLK_CH
                    it = wpool.tile([P, CHS // 16], I16, tag=f"idx{r}{b}", name=f"idx{r}{b}", bufs=3)
                    nc.sync.dma_start(
                        out=it[:, :nblk * P // 16],
                        in_=idxt[:, gblk0 * P // 16:(gblk0 + nblk) * P // 16])
                    mt = wpool.tile([P, NBLK_CH], BF16, tag=f"meta{r}{b}", name=f"meta{r}{b}", bufs=3)
                    nc.sync.dma_start(
                        out=mt[:, :nblk],
                        in_=dwbt[:, gblk0:gblk0 + nblk])
                    gt = wpool.tile([P, NBLK_CH, P], BF16, tag=f"st{r}{b}", name=f"st{r}{b}", bufs=3)
                    nc.gpsimd.dma_gather(
                        out_ap=gt[:, :nblk, :],
                        in_ap=table[b * BR:min((b + 1) * BR, TROWS), :],
                        idxs_ap=it[:, :nblk * P // 16],
                        num_idxs=nblk * P, num_idxs_reg=nblk * P,
                        elem_size=H, single_packet=False,
                        queue_num=qctr[0] % NSWQ)
                    qctr[0] += 1
                    cur[key] = (ch, gt, mt)
                    return cur[key]

                for w in range(NW):
                    ws = slice(w * WIN, (w + 1) * WIN)
                    # per-dst mean reciprocals for both relations, broadcast
                    # across partitions via PE outer product with ones
                    brcp = ppool.tile([P, R, WIN], F32, tag="brcp",
                                      name="brcp", space="PSUM", bufs=1)
                    for r in range(R):
                        nc.tensor.matmul(
                            out=brcp[:, r, :], lhsT=ones1[:],
                            rhs=recip_t[0:1, r * SHP + w * WIN:
                                        r * SHP + (w + 1) * WIN],
                            start=True, stop=True)
                    brc = wpool.tile([P, R, WIN], BF16, tag="brc", name="brc",
                                     bufs=3)
                    nc.scalar.copy(out=brc[:], in_=brcp[:])
                    agg = []
                    for r in range(R):
                        pa = ppool.tile([P, WIN], F32, tag=f"agg{r}", name=f"agg{r}",
                                        space="PSUM")
                        nblk_w = int(pl.bpg[r, :, w].sum())
                        j = 0
                        for b in range(BANKS):
                            base = int(pl.group_blk_base[r, b, w]
                                       - pl.stream_blk_base[r, b])
                            for k in range(int(pl.bpg[r, b, w])):
                                blkloc = base + k
                                ch, gt, mt = ensure_chunk(r, b, blkloc)
                                pos = blkloc - ch * NBLK_CH
                                st = wpool.tile([P, WIN], BF16, tag="s", name="s", bufs=16)
                                nc.vector.tensor_tensor(
                                    out=st[:], in0=iota[:],
                                    in1=mt[:, pos:pos + 1].to_broadcast(
                                        [P, WIN]),
                                    op=mybir.AluOpType.is_equal)
                                nc.tensor.matmul(
                                    out=pa[:], lhsT=gt[:, pos, :], rhs=st[:],
                                    start=(j == 0), stop=(j == nblk_w - 1))
                                j += 1
                        asb = wpool.tile([P, WIN], BF16, tag=f"asb{r}", name=f"asb{r}", bufs=4)
                        if nblk_w == 0:
                            nc.vector.memset(asb[:], 0.0)
                        else:
                            # mean: scale raw sums by per-dst reciprocal count
                            nc.vector.tensor_tensor(
                                out=asb[:], in0=pa[:], in1=brc[:, r, :],
                                op=mybir.AluOpType.mult)
                        agg.append(asb)

                    po = ppool.tile([P, WIN], F32, tag="po", name="po", space="PSUM", bufs=2)
                    nc.tensor.matmul(out=po[:], lhsT=rootw[:, 0, :],
                                     rhs=h_in[:, ws], start=True, stop=False)
                    for r in range(R):
                        nc.tensor.matmul(out=po[:], lhsT=relw[r][:, 0, :],
                                         rhs=agg[r][:], start=False,
                                         stop=(r == R - 1))
                    nc.scalar.activation(
                        out=h_out[:, ws], in_=po[:],
                        func=mybir.ActivationFunctionType.Identity,
                        bias=bias_t[:, bias_col:bias_col + 1])
                    if table_out is not None:
                        emit_table_slice(h_out, table_out, w * WIN,
                                         (w + 1) * WIN, pool=tpool)
                    if cls and (w + 1) % CB == 0 and w + 1 >= 2 * CB:
                        emit_cls(w + 1 - CB)
                if cls:
                    emit_cls(NW)


            # layer 1 (emits layer-2 table slices as windows complete)
            emit_layer(0, hT[0], hT[1], cc_out[0],
                       wt["root1"], [wt["rel10"], wt["rel11"]], 6,
                       table_out=cc_in[1])
            nc.gpsimd.collective_compute(
                "AllGather", mybir.AluOpType.bypass,
                ins=[cc_in[1][:]], outs=[cc_out[1][:]], replica_groups=rg)
            # layer 2 (classifier interleaved in lagged batches)
            emit_layer(1, hT[1], hT[2], cc_out[1],
                       wt["root2"], [wt["rel20"], wt["rel21"]], 7, cls=True)

    nc.compile()
    return nc


# ---------------------------------------------------------------------------
# entry point
# ---------------------------------------------------------------------------

def kernel(**inputs):
    cfg = _derived(CFG)
    return _kernel_impl(inputs, cfg)


def _kernel_impl(inputs, cfg, trace=False):
    d = cfg
    NC, SH, SHP = d["NC"], d["SH"], d["SHP"]

    pl = build_plan(inputs["edge_index"], inputs["edge_type"], d)
    xs = prep_x(np.asarray(inputs["x"], np.float32), pl, d)
    w = prep_weights(inputs, d)

    nc = build_bass(d, pl)

    in_maps = []
    for c in range(NC):
        m = {"xT": xs[c], "idxt": pl.idx16[c], "dwbt": pl.dwb[c],
             "recipt": pl.recip[c], "biases": w["biases"]}
        for nm in ["wdes", "wtweet", "wnum", "wcat", "win", "root1", "rel10",
                   "rel11", "root2", "rel20", "rel21", "wcls"]:
            m[nm] = w[nm]
        in_maps.append(m)

    res = run_bass_kernel_spmd(nc, in_maps, core_ids=list(range(NC)),
                               trace=trace)

    out = np.empty((NC * SH, d["H"]), np.float32)
    for c in range(NC):
        out[c * SH:(c + 1) * SH] = res.results[c]["outT"].T[pl.node_slot[c]]
    if trace:
        return out, res
    return out


# revision 50
# speedup vs baseline: 1.0397x; 1.0397x over previous
"""BotRGCN Trainium2 kernel: feature transform + 2 RGCN layers + classifier.

Sharding: nodes split across 8 cores by id (12500/core, padded to 12544).
Within each core, nodes are assigned to 256-wide dst windows by a balancing
greedy (equalizes per-(rel, src-bank, window) edge counts, minimizing block
padding). Edges partitioned by destination shard; per (relation, dst-window,
src-bank) groups padded to a block structure uniform across cores so a single
SPMD program serves all 8 cores. Source features exchanged via bf16 AllGather
of the per-layer node-feature table; gathers via int16 dma_gather per src
bank, round-robined across 4 SWDGE queues.
"""

import sys

sys.path.insert(0, "/opt/trn_rl_repo")

from contextlib import ExitStack

import numpy as np
import ml_dtypes

import concourse.bass as bass
import concourse.bacc as bacc
import concourse.mybir as mybir
import concourse.tile as tile
from concourse.masks import make_identity
from concourse.bass_utils import run_bass_kernel_spmd

BF16 = mybir.dt.bfloat16
F32 = mybir.dt.float32
I16 = mybir.dt.int16

P = 128

# full-problem config (test.py overrides for mini runs)
CFG = dict(
    N=100000,        # nodes
    NC=8,            # cores
    R=2,             # relations
    H=128,
    DES=768, TWEET=768, NUMP=6, CATP=11,
    WIN=128,         # dst window (PSUM free dim)
    NBLK_CH=12,      # gather-chunk size in 128-edge blocks
    EXTRA_WIN=2,     # slack windows so groups fit their ceil block count
    NTF=512,         # feature-stage node tile
    NSWQ=4,          # SWDGE queues for gather desc-gen
)


def _derived(cfg):
    d = dict(cfg)
    d["SH"] = cfg["N"] // cfg["NC"]
    shp0 = ((d["SH"] + P - 1) // P) * P
    shp0 = ((shp0 + cfg["WIN"] - 1) // cfg["WIN"]) * cfg["WIN"]
    d["SHP"] = shp0 + cfg["EXTRA_WIN"] * cfg["WIN"]
    d["NW"] = d["SHP"] // cfg["WIN"]
    d["BANKROWS"] = 2 * d["SHP"]                # bank == src core pair
    assert d["BANKROWS"] < 2 ** 15
    d["TROWS"] = cfg["NC"] * d["SHP"]           # padded table rows
    d["BANKS"] = (d["TROWS"] + d["BANKROWS"] - 1) // d["BANKROWS"]
    d["TBLK"] = d["SHP"] // P                   # 128-row blobs per core
    # x feature layout: [des | tweet | num(pad to 128) | cat(pad to 128)]
    d["KDES"] = cfg["DES"] // P
    d["KTWEET"] = cfg["TWEET"] // P
    d["KX"] = d["KDES"] + d["KTWEET"] + 2
    d["XROWS"] = d["KX"] * P
    return d


# ---------------------------------------------------------------------------
# host-side graph planning
# ---------------------------------------------------------------------------

class Plan:
    pass


def _balance_windows(deg, NW, WIN, cap):
    """Greedy node->window assignment equalizing per-dim load.

    deg: [SH, D] per-node degree vectors; cap: [D] per-dim soft cap per
    window (whole blocks). Returns slot id per node (window w slots are
    [w*WIN, (w+1)*WIN))."""
    SH = deg.shape[0]
    order = np.argsort(-deg.sum(1), kind="stable")
    load = np.zeros((NW, deg.shape[1]), np.int64)
    nslots = np.zeros(NW, np.int64)
    slot = np.zeros(SH, np.int64)
    for v in order:
        d = deg[v]
        nl = load + d
        over = np.maximum(nl - cap, 0).sum(1).astype(np.float64)
        sc = over * 1e9 + nl.max(1) + nslots * 1e-6
        sc[nslots >= WIN] = 1e18
        w = int(np.argmin(sc))
        slot[v] = w * WIN + nslots[w]
        load[w] += d
        nslots[w] += 1
    return slot


def build_plan(edge_index, edge_type, cfg):
    """Group edges per core by (rel, dst-window, src-bank); pad each group to a
    whole number of 128-edge blocks, uniform across cores. Returns per-core
    gather-index / meta arrays plus the uniform block structure."""
    d = cfg
    NC, SH, SHP, WIN, NW = d["NC"], d["SH"], d["SHP"], d["WIN"], d["NW"]
    BANKS, BR, NBLK_CH = d["BANKS"], d["BANKROWS"], d["NBLK_CH"]
    R = d["R"]
    N = d["N"]
    TBLK = d["TBLK"]

    src = np.asarray(edge_index[0], dtype=np.int64)
    dst = np.asarray(edge_index[1], dtype=np.int64)
    et = np.asarray(edge_type, dtype=np.int64)

    core = dst // SH
    dl = dst - core * SH
    score = src // SH                 # src core
    sbank = score // max(1, (BR // SHP))   # src bank == src core pair

    # --- balanced window assignment per core (slot of each local node) ---
    D = R * BANKS
    node_slot = np.zeros((NC, SH), np.int64)
    for c in range(NC):
        m = core == c
        deg = np.zeros((SH, D), np.int32)
        np.add.at(deg, (dl[m], et[m] * BANKS + sbank[m]), 1)
        cap = ((np.ceil(deg.sum(0) / NW / P)) * P).astype(np.int64)
        node_slot[c] = _balance_windows(deg, NW, WIN, cap)

    # dst slot / window / in-window pos
    dslot = node_slot[core, dl]
    win = dslot // WIN
    dw = (dslot - win * WIN).astype(np.float32)

    # src table row: slot-major (row = core*SHP + slot)
    sl = node_slot[score, src - score * SH]
    ps = score * SHP + sl
    bank = ps // BR
    bidx = (ps - bank * BR).astype(np.int16)
    assert np.all(bank == sbank)

    # per-(rel, node) in-degree -> per-edge mean weight
    cnt = np.bincount(et * N + dst, minlength=R * N).reshape(R, N)
    wv = (1.0 / np.maximum(cnt, 1.0))[et, dst].astype(np.float32)

    # group = (rel, bank, win); uniform block counts = max over cores
    NG = R * BANKS * NW
    gid = (et * BANKS + bank) * NW + win
    counts = np.bincount(core * NG + gid, minlength=NC * NG).reshape(NC, NG)
    bpg = (counts.max(axis=0) + P - 1) // P            # blocks per group
    bpg = bpg.reshape(R, BANKS, NW).copy()

    slots_per_group = (bpg.reshape(-1) * P)
    slot_base = np.zeros(NG + 1, np.int64)
    np.cumsum(slots_per_group, out=slot_base[1:])
    TOTSLOT = int(slot_base[-1])
    TOTBLK = TOTSLOT // P

    # place each edge into its group's slot range (per core)
    okey = core * NG + gid
    order = np.argsort(okey, kind="stable")
    so = okey[order]
    first_of = np.r_[True, so[1:] != so[:-1]]
    idx_in_run = np.arange(len(so)) - np.maximum.accumulate(
        np.where(first_of, np.arange(len(so)), 0)
    )
    slot = slot_base[so % NG] + idx_in_run

    # per-block dst-in-window position (bf16, -1 marks pad slots)
    idx16 = np.zeros((NC, 8 * 16, TOTSLOT // 16), np.int16)
    dwb = np.full((NC, P, TOTBLK), -1.0, ml_dtypes.bfloat16)
    ecore = core[order]
    col = slot // 16
    prow = (slot % 16).astype(np.int64)
    for g in range(8):
        idx16[ecore, 16 * g + prow, col] = bidx[order]
    dwb[ecore, slot % P, slot // P] = dw[order]

    # per-(rel, dst-slot) mean reciprocal, bf16 row vector per core
    recip = np.ones((NC, 1, R * SHP), np.float32)
    for c in range(NC):
        nd = np.arange(SH) + c * SH
        for r in range(R):
            recip[c, 0, r * SHP + node_slot[c]] = \
                1.0 / np.maximum(cnt[r, nd], 1.0)
    recip = recip.astype(ml_dtypes.bfloat16)

    pl = Plan()
    pl.idx16 = idx16.reshape(NC, P, TOTSLOT // 16)
    pl.dwb = dwb
    pl.recip = recip
    pl.bpg = bpg
    pl.TOTBLK = TOTBLK
    pl.node_slot = node_slot
    # stream bookkeeping: stream s = r*BANKS+b; block base per stream; block
    # base per group within stream
    pl.group_blk_base = np.zeros((R, BANKS, NW), np.int64)
    base = 0
    pl.stream_blk_base = np.zeros((R, BANKS), np.int64)
    pl.stream_nblk = np.zeros((R, BANKS), np.int64)
    for r in range(R):
        for b in range(BANKS):
            pl.stream_blk_base[r, b] = base
            for w in range(NW):
                pl.group_blk_base[r, b, w] = base
                base += int(bpg[r, b, w])
            pl.stream_nblk[r, b] = base - pl.stream_blk_base[r, b]
    assert base == TOTBLK
    return pl


def prep_x(x, pl, cfg):
    """Per-core transposed bf16 feature blocks [XROWS, SHP], columns in slot
    order."""
    d = cfg
    NC, SH, SHP = d["NC"], d["SH"], d["SHP"]
    NUMP, TWEET, CATP, DES = d["NUMP"], d["TWEET"], d["CATP"], d["DES"]
    KD, KT = d["KDES"], d["KTWEET"]
    out = np.zeros((NC, d["XROWS"], SHP), ml_dtypes.bfloat16)
    for c in range(NC):
        xs = x[c * SH:(c + 1) * SH]
        xT = np.zeros((d["XROWS"], SHP), np.float32)
        sl = pl.node_slot[c]
        xT[:DES, sl] = xs[:, NUMP + TWEET + CATP:].T
        xT[DES:DES + TWEET, sl] = xs[:, NUMP:NUMP + TWEET].T
        xT[(KD + KT) * P:(KD + KT) * P + NUMP, sl] = xs[:, :NUMP].T
        xT[(KD + KT + 1) * P:(KD + KT + 1) * P + CATP, sl] = \
            xs[:, NUMP + TWEET:NUMP + TWEET + CATP].T
        out[c] = xT.astype(ml_dtypes.bfloat16)
    return out


def prep_weights(inp, cfg):
    """bf16 weight blocks + packed fp32 biases."""
    bf = lambda a: np.asarray(a, np.float32).astype(ml_dtypes.bfloat16)
    d = cfg
    wnum = np.zeros((P, d["H"]), np.float32)
    wnum[:d["NUMP"]] = inp["W_num"]
    wcat = np.zeros((P, d["H"]), np.float32)
    wcat[:d["CATP"]] = inp["W_cat"]
    w = {
        "wdes": bf(inp["W_des"]), "wtweet": bf(inp["W_tweet"]),
        "wnum": bf(wnum), "wcat": bf(wcat), "win": bf(inp["W_in"]),
        "root1": bf(inp["root1"]), "rel10": bf(inp["rel1"][0]),
        "rel11": bf(inp["rel1"][1]),
        "root2": bf(inp["root2"]), "rel20": bf(inp["rel2"][0]),
        "rel21": bf(inp["rel2"][1]), "wcls": bf(inp["W_cls"]),
    }
    biases = np.stack(
        [inp["b_des"], inp["b_tweet"], inp["b_num"], inp["b_cat"],
         inp["b_in"], inp["prelu_a"], inp["bias1"], inp["bias2"],
         inp["b_cls"]], axis=1).astype(np.float32)   # [128, 9]
    w["biases"] = biases
    return w


# ---------------------------------------------------------------------------
# bass program
# ---------------------------------------------------------------------------

def build_bass(cfg, pl):
    d = cfg
    NC, SHP, WIN, NW, NTF = d["NC"], d["SHP"], d["WIN"], d["NW"], d["NTF"]
    BANKS, BR, NBLK_CH = d["BANKS"], d["BANKROWS"], d["NBLK_CH"]
    R, H = d["R"], d["H"]
    KD, KT, KX = d["KDES"], d["KTWEET"], d["KX"]
    TBLK = d["TBLK"]
    TROWS = d["TROWS"]
    NSWQ = d["NSWQ"]
    CHS = NBLK_CH * P      # idx slots per chunk

    nc = bacc.Bacc(None, target_bir_lowering=False, debug=False,
                   num_devices=NC, num_swdge_queues=NSWQ)

    # ---- I/O ----
    xT = nc.dram_tensor("xT", [d["XROWS"], SHP], BF16, kind="ExternalInput")
    idxt = nc.dram_tensor("idxt", [P, pl.TOTBLK * P // 16], I16, kind="ExternalInput")
    dwbt = nc.dram_tensor("dwbt", [P, pl.TOTBLK], BF16, kind="ExternalInput")
    recipt = nc.dram_tensor("recipt", [1, R * SHP], BF16, kind="ExternalInput")
    wts = {}
    for nm, shp in [("wdes", [d["DES"], H]), ("wtweet", [d["TWEET"], H]),
                    ("wnum", [P, H]), ("wcat", [P, H]), ("win", [4 * P, H]),
                    ("root1", [H, H]), ("rel10", [H, H]), ("rel11", [H, H]),
                    ("root2", [H, H]), ("rel20", [H, H]), ("rel21", [H, H]),
                    ("wcls", [H, H])]:
        wts[nm] = nc.dram_tensor(nm, shp, BF16, kind="ExternalInput")
    biases = nc.dram_tensor("biases", [P, 9], F32, kind="ExternalInput")
    outT = nc.dram_tensor("outT", [P, SHP], F32, kind="ExternalOutput")

    # ---- collective tables ----
    cc_in = [nc.dram_tensor(f"cc{i}_in", [SHP, H], BF16, kind="Internal")
             for i in (1, 2)]
    cc_out = [nc.dram_tensor(f"cc{i}_out", [NC * SHP, H], BF16,
                             kind="Internal", addr_space="Shared")
              for i in (1, 2)]

    rg = [list(range(NC))]
    qctr = [0]  # SWDGE queue rotation

    def cc_views(cc_in_t, cc_out_t, row0, row1):
        """Input slice + rank-strided output view for a chunked AllGather."""
        iv = cc_in_t[row0:row1, :]
        ov = cc_out_t.rearrange("(c s) h -> c s h", c=NC)[:, row0:row1, :]
        return iv, ov

    with tile.TileContext(nc) as tc:
        with (
            tc.tile_pool(name="const", bufs=1) as cpool,
            tc.tile_pool(name="resident", bufs=1) as rpool,
            ExitStack() as mstack,
        ):
            # ---- constants ----
            ident = cpool.tile([P, P], BF16)
            make_identity(nc, ident[:])
            iota = cpool.tile([P, WIN], BF16)
            nc.gpsimd.iota(iota[:], pattern=[[1, WIN]], base=0,
                           channel_multiplier=0,
                           allow_small_or_imprecise_dtypes=True)
            bias_t = cpool.tile([P, 9], F32)
            nc.sync.dma_start(out=bias_t[:], in_=biases[:])
            recip_t = cpool.tile([1, R * SHP], BF16)
            nc.sync.dma_start(out=recip_t[:], in_=recipt[:])
            ones1 = cpool.tile([1, P], BF16)
            nc.vector.memset(ones1[:], 1.0)

            wt = {}
            for nm, kb in [("wdes", KD), ("wtweet", KT), ("wnum", 1),
                           ("wcat", 1), ("win", 4), ("root1", 1),
                           ("rel10", 1), ("rel11", 1), ("root2", 1),
                           ("rel20", 1), ("rel21", 1), ("wcls", 1)]:
                t = cpool.tile([P, kb, H], BF16, tag=f"w_{nm}", name=f"w_{nm}")
                nc.sync.dma_start(
                    out=t[:], in_=wts[nm].rearrange("(k p) h -> p k h", p=P))
                wt[nm] = t

            # resident activations (transposed, [H, SHP] bf16)
            hT = [rpool.tile([P, SHP], BF16, tag="ht", name=f"hT{i}", bufs=2)
                  for i in range(3)]

            spool = mstack.enter_context(tc.tile_pool(name="tstage", bufs=4))

            def emit_table_slice(src_hT, cc_in_t, n0, n1, on_vec=False,
                                 pool=None, tbufs=1):
                """Transpose hT[:, n0:n1] into node-major rows + DMA to
                cc_in."""
                b0, b1 = n0 // P, n1 // P
                stage = spool.tile([P, 4, P], BF16, tag="tstage", name="tstage")
                for blk in range(b0, b1):
                    tp = pool.tile([P, P], BF16, tag="tp", name="tp",
                                   space="PSUM", bufs=tbufs)
                    nc.tensor.transpose(
                        out=tp[:], in_=src_hT[:, blk * P:(blk + 1) * P],
                        identity=ident[:])
                    if on_vec:
                        nc.vector.tensor_copy(out=stage[:, blk - b0, :],
                                              in_=tp[:])
                    else:
                        nc.scalar.copy(out=stage[:, blk - b0, :], in_=tp[:])
                nc.sync.dma_start(
                    out=cc_in_t.rearrange("(t p) h -> p t h", p=P)[:, b0:b1],
                    in_=stage[:, :b1 - b0])

            # =============== feature transform ===============
            fstack = ExitStack()
            fpool = fstack.enter_context(tc.tile_pool(name="featsb", bufs=4))
            fpp = fstack.enter_context(
                tc.tile_pool(name="featps", bufs=2, space="PSUM"))
            ntiles = (SHP + NTF - 1) // NTF
            for t in range(ntiles):
                n0 = t * NTF
                n1 = min(SHP, n0 + NTF)
                nn = n1 - n0
                xt = fpool.tile([P, KX, NTF], BF16, tag="xt", name="xt")
                xv = xT.rearrange("(k p) n -> p k n", p=P)
                # per-branch loads so branch matmuls start as columns arrive
                for ks, kn in [(0, KD), (KD, KT), (KD + KT, 2)]:
                    nc.sync.dma_start(
                        out=xt[:, ks:ks + kn, :nn],
                        in_=xv[:, ks:ks + kn, n0:n1])

                zb = []
                for bi, (wnm, ks, kn) in enumerate([
                        ("wdes", 0, KD), ("wtweet", KD, KT),
                        ("wnum", KD + KT, 1), ("wcat", KD + KT + 1, 1)]):
                    pz = fpp.tile([P, NTF], F32, tag=f"pz{bi}", name=f"pz{bi}", space="PSUM", bufs=1)
                    for k in range(kn):
                        nc.tensor.matmul(
                            out=pz[:, :nn], lhsT=wt[wnm][:, k, :],
                            rhs=xt[:, ks + k, :nn],
                            start=(k == 0), stop=(k == kn - 1))
                    v = fpool.tile([P, NTF], BF16, tag=f"v{bi}", name=f"v{bi}")
                    nc.scalar.activation(
                        out=v[:, :nn], in_=pz[:, :nn],
                        func=mybir.ActivationFunctionType.Identity,
                        bias=bias_t[:, bi:bi + 1])
                    z = fpool.tile([P, NTF], BF16, tag=f"z{bi}", name=f"z{bi}")
                    nc.vector.scalar_tensor_tensor(
                        out=z[:, :nn], in0=v[:, :nn], scalar=0.01,
                        in1=v[:, :nn], op0=mybir.AluOpType.mult,
                        op1=mybir.AluOpType.max)
                    zb.append(z)

                ph = fpp.tile([P, NTF], F32, tag="ph", name="ph", space="PSUM")
                for k in range(4):
                    nc.tensor.matmul(out=ph[:, :nn], lhsT=wt["win"][:, k, :],
                                     rhs=zb[k][:, :nn],
                                     start=(k == 0), stop=(k == 3))
                vh = fpool.tile([P, NTF], F32, tag="vh", name="vh")
                nc.scalar.activation(
                    out=vh[:, :nn], in_=ph[:, :nn],
                    func=mybir.ActivationFunctionType.Identity,
                    bias=bias_t[:, 4:5])
                nc.vector.scalar_tensor_tensor(
                    out=hT[0][:, n0:n1], in0=vh[:, :nn],
                    scalar=bias_t[:, 5:6], in1=vh[:, :nn],
                    op0=mybir.AluOpType.mult, op1=mybir.AluOpType.max)
                emit_table_slice(hT[0], cc_in[0], n0, n1, on_vec=True,
                                 pool=fpp, tbufs=2)

            fstack.close()
            tpool = mstack.enter_context(
                tc.tile_pool(name="tps", bufs=2, space="PSUM"))
            nc.gpsimd.collective_compute(
                "AllGather", mybir.AluOpType.bypass,
                ins=[cc_in[0][:]], outs=[cc_out[0][:]], replica_groups=rg)

            wpool = mstack.enter_context(tc.tile_pool(name="work", bufs=3))
            ppool = mstack.enter_context(
                tc.tile_pool(name="psum", bufs=2, space="PSUM"))

            # =============== per-layer helper ===============
            def emit_layer(li, h_in, h_out, table, rootw, relw, bias_col,
                           table_out=None, cls=False):
                # per-stream gather state
                cur = {}

                def ensure_chunk(r, b, blkloc):
                    ch = blkloc // NBLK_CH
                    key = (r, b)
                    if cur.get(key, (-1,))[0] == ch:
                        return cur[key]
                    nblk = min(NBLK_CH,
                               int(pl.stream_nblk[r, b]) - ch * NBLK_CH)
                    gblk0 = int(pl.stream_blk_base[r, b]) + ch * NBLK_CH
                    it = wpool.tile([P, CHS // 16], I16, tag=f"idx{r}{b}", name=f"idx{r}{b}", bufs=3)
                    nc.sync.dma_start(
                        out=it[:, :nblk * P // 16],
                        in_=idxt[:, gblk0 * P // 16:(gblk0 + nblk) * P // 16])
                    mt = wpool.tile([P, NBLK_CH], BF16, tag=f"meta{r}{b}", name=f"meta{r}{b}", bufs=3)
                    nc.sync.dma_start(
                        out=mt[:, :nblk],
                        in_=dwbt[:, gblk0:gblk0 + nblk])
                    gt = wpool.tile([P, NBLK_CH, P], BF16, tag=f"st{r}{b}", name=f"st{r}{b}", bufs=3)
                    nc.gpsimd.dma_gather(
                        out_ap=gt[:, :nblk, :],
                        in_ap=table[b * BR:min((b + 1) * BR, TROWS), :],
                        idxs_ap=it[:, :nblk * P // 16],
                        num_idxs=nblk * P, num_idxs_reg=nblk * P,
                        elem_size=H, single_packet=False,
                        queue_num=qctr[0] % NSWQ)
                    qctr[0] += 1
                    cur[key] = (ch, gt, mt)
                    return cur[key]

                for w in range(NW):
                    ws = slice(w * WIN, (w + 1) * WIN)
                    # per-dst mean reciprocals for both relations, broadcast
                    # across partitions via PE outer product with ones
                    brcp = ppool.tile([P, R, WIN], F32, tag="brcp",
                                      name="brcp", space="PSUM", bufs=1)
                    for r in range(R):
                        nc.tensor.matmul(
                            out=brcp[:, r, :], lhsT=ones1[:],
                            rhs=recip_t[0:1, r * SHP + w * WIN:
                                        r * SHP + (w + 1) * WIN],
                            start=True, stop=True)
                    brc = wpool.tile([P, R, WIN], BF16, tag="brc", name="brc",
                                     bufs=3)
                    nc.scalar.copy(out=brc[:], in_=brcp[:])
                    agg = []
                    for r in range(R):
                        pa = ppool.tile([P, WIN], F32, tag=f"agg{r}", name=f"agg{r}",
                                        space="PSUM")
                        nblk_w = int(pl.bpg[r, :, w].sum())
                        j = 0
                        for b in range(BANKS):
                            base = int(pl.group_blk_base[r, b, w]
                                       - pl.stream_blk_base[r, b])
                            for k in range(int(pl.bpg[r, b, w])):
                                blkloc = base + k
                                ch, gt, mt = ensure_chunk(r, b, blkloc)
                                pos = blkloc - ch * NBLK_CH
                                st = wpool.tile([P, WIN], BF16, tag="s", name="s", bufs=16)
                                nc.vector.tensor_tensor(
                                    out=st[:], in0=iota[:],
                                    in1=mt[:, pos:pos + 1].to_broadcast(
                                        [P, WIN]),
                                    op=mybir.AluOpType.is_equal)
                                nc.tensor.matmul(
                                    out=pa[:], lhsT=gt[:, pos, :], rhs=st[:],
                                    start=(j == 0), stop=(j == nblk_w - 1))
                                j += 1
                        asb = wpool.tile([P, WIN], BF16, tag=f"asb{r}", name=f"asb{r}", bufs=4)
                        if nblk_w == 0:
                            nc.vector.memset(asb[:], 0.0)
                        else:
                            # mean: scale raw sums by per-dst reciprocal count
                            nc.vector.tensor_tensor(
                                out=asb[:], in0=pa[:], in1=brc[:, r, :],
                                op=mybir.AluOpType.mult)
                        agg.append(asb)

                    po = ppool.tile([P, WIN], F32, tag="po", name="po", space="PSUM", bufs=2)
                    nc.tensor.matmul(out=po[:], lhsT=rootw[:, 0, :],
                                     rhs=h_in[:, ws], start=True, stop=False)
                    for r in range(R):
                        nc.tensor.matmul(out=po[:], lhsT=relw[r][:, 0, :],
                                         rhs=agg[r][:], start=False,
                                         stop=(r == R - 1))
                    nc.scalar.activation(
                        out=h_out[:, ws], in_=po[:],
                        func=mybir.ActivationFunctionType.Identity,
                        bias=bias_t[:, bias_col:bias_col + 1])
                    if table_out is not None:
                        emit_table_slice(h_out, table_out, w * WIN,
                                         (w + 1) * WIN, pool=tpool)


            # layer 1 (emits layer-2 table slices as windows complete)
            emit_layer(0, hT[0], hT[1], cc_out[0],
                       wt["root1"], [wt["rel10"], wt["rel11"]], 6,
                       table_out=cc_in[1])
            nc.gpsimd.collective_compute(
                "AllGather", mybir.AluOpType.bypass,
                ins=[cc_in[1][:]], outs=[cc_out[1][:]], replica_groups=rg)
            # layer 2
            emit_layer(1, hT[1], hT[2], cc_out[1],
                       wt["root2"], [wt["rel20"], wt["rel21"]], 7)

            # =============== classifier ===============
            # alternate PSUM tags (agg0/agg1 are dead after layer 2) to widen
            # the effective ring and shorten the pc->act chain
            for w in range(NW):
                ws = slice(w * WIN, (w + 1) * WIN)
                tag = ("po", "agg0", "agg1")[w % 3]
                pc = ppool.tile([P, WIN], F32, tag=tag, name="pc",
                                space="PSUM", bufs=2)
                nc.tensor.matmul(out=pc[:], lhsT=wt["wcls"][:, 0, :],
                                 rhs=hT[2][:, ws], start=True, stop=True)
                oc = wpool.tile([P, WIN], F32, tag="oc", name="oc", bufs=4)
                nc.scalar.activation(
                    out=oc[:], in_=pc[:],
                    func=mybir.ActivationFunctionType.Identity,
                    bias=bias_t[:, 8:9])
                nc.sync.dma_start(out=outT[:, ws], in_=oc[:])

    nc.compile()
    return nc


# ---------------------------------------------------------------------------
# entry point
# ---------------------------------------------------------------------------

def kernel(**inputs):
    cfg = _derived(CFG)
    return _kernel_impl(inputs, cfg)


def _kernel_impl(inputs, cfg, trace=False):
    d = cfg
    NC, SH, SHP = d["NC"], d["SH"], d["SHP"]

    pl = build_plan(inputs["edge_index"], inputs["edge_type"], d)
    xs = prep_x(np.asarray(inputs["x"], np.float32), pl, d)
    w = prep_weights(inputs, d)

    nc = build_bass(d, pl)

    in_maps = []
    for c in range(NC):
        m = {"xT": xs[c], "idxt": pl.idx16[c], "dwbt": pl.dwb[c],
             "recipt": pl.recip[c], "biases": w["biases"]}
        for nm in ["wdes", "wtweet", "wnum", "wcat", "win", "root1", "rel10",
                   "rel11", "root2", "rel20", "rel21", "wcls"]:
            m[nm] = w[nm]
        in_maps.append(m)

    res = run_bass_kernel_spmd(nc, in_maps, core_ids=list(range(NC)),
                               trace=trace)

    out = np.empty((NC * SH, d["H"]), np.float32)
    for c in range(NC):
        out[c * SH:(c + 1) * SH] = res.results[c]["outT"].T[pl.node_slot[c]]
    if trace:
        return out, res
    return out
